# revision 18
# baseline (speedup 1.0000x reference)
"""BiQRNN forward kernel for Trainium2 (8 NeuronCores, batch-sharded).

Model (see reference):
  ev  = X[:,:,0] (int ids), num = X[:,:,1:]
  e   = emb[ev]                      [B,S,256]
  n   = num @ Wn + bn                [B,S,4]
  c   = [e, n]                       [B,S,260]
  g   = c @ W + b  (W in {Wf,Wb})    -> Z = tanh(g[:,:512]), F = sigmoid(g[:,512:1024])
  hf  = fo_pool(Zf,Ff)[-1]  (h_t = F h_{t-1} + (1-F) Z)
  hb  = (1-Fb[S-1]) * Zb[S-1]        (only last step of reversed scan survives)
  out = [hf, hb] @ Wo + bo           [B,1]

Device strategy per core (8 batches each):
  - one dma_gather(transpose=True) family delivers emb rows as e^T [128,2,4096] bf16
  - gate GEMM computed transposed: G^T[h, tok] = W^T ... via matmul(lhsT=W, rhs=e^T)
    3 K-passes: emb dims 0:128, 128:256, then [num(7) + ones(1)] with Wn/bias folded in
  - scalar engine drains PSUM through tanh/sigmoid
  - w~ = (s-1)*z via scalar_tensor_tensor; h via tensor_tensor_scan: st = s*st - w~
  - output projection via small accumulating matmuls (Wo backward half pre-negated)
"""
import numpy as np

import concourse.bacc as bacc
import concourse.bass as bass
import concourse.mybir as mybir
import concourse.tile as tile
from concourse import bass_utils

F32 = mybir.dt.float32
BF16 = mybir.dt.bfloat16
I16 = mybir.dt.int16
I32 = mybir.dt.int32
NP_BF16 = mybir.dt.np(BF16)

VOCAB, EMB, HID, OUT = 1000, 256, 512, 1
NUM_IN, NUM_OUT = 7, 4
B, S = 64, 512
NCORES = 8
BC = B // NCORES          # 8 batches per core
NT = BC * S               # 4096 tokens per core
AF = mybir.ActivationFunctionType
ALU = mybir.AluOpType

# engine placement knobs (tuned via profiling)
SCAN_ENGINES = ["vector", "vector", "vector", "vector"]  # per j-chunk
STT_ENGINE = "vector"
N_GATHER_CHUNKS = 4


def build_kernel(debug=False):
    nc = bacc.Bacc("TRN2", target_bir_lowering=False, debug=debug)

    idx_d = nc.dram_tensor("idx32", [128, NT // 128], I32, kind="ExternalInput")
    numt1_d = nc.dram_tensor("numt1", [NUM_IN + 1, NT], BF16, kind="ExternalInput")
    emb_d = nc.dram_tensor("emb", [VOCAB, EMB], BF16, kind="ExternalInput")
    wf_d = nc.dram_tensor("wf", [128, 2 * 2 * HID], BF16, kind="ExternalInput")
    wnfb_d = nc.dram_tensor("wnfb", [NUM_IN + 1, 2 * HID], BF16, kind="ExternalInput")
    wb_d = nc.dram_tensor("wb", [128, 2 * 2 * HID], BF16, kind="ExternalInput")
    wnbb_d = nc.dram_tensor("wnbb", [NUM_IN + 1, 2 * HID], BF16, kind="ExternalInput")
    wo_d = nc.dram_tensor("wo", [128, 8], F32, kind="ExternalInput")
    bo_d = nc.dram_tensor("bo", [1, 1], BF16, kind="ExternalInput")
    out_d = nc.dram_tensor("out", [BC, 1], F32, kind="ExternalOutput")

    def eng(name):
        return {"vector": nc.vector, "gpsimd": nc.gpsimd}[name]

    with tile.TileContext(nc) as tc:
        with tc.tile_pool(name="const", bufs=1) as cpool, \
             tc.tile_pool(name="work", bufs=2) as wpool, \
             tc.tile_pool(name="ps", bufs=3, space="PSUM") as ps, \
             tc.tile_pool(name="pst", bufs=2, space="PSUM") as pst:
            # ---- constant loads ----
            idx_sb = cpool.tile([128, NT // 128], I32)
            nc.sync.dma_start(out=idx_sb[:], in_=idx_d[:])
            wf_sb = cpool.tile([128, 2048], BF16)
            nc.sync.dma_start(out=wf_sb[:], in_=wf_d[:])
            wb_sb = cpool.tile([128, 2048], BF16)
            nc.sync.dma_start(out=wb_sb[:], in_=wb_d[:])
            wnfb_sb = cpool.tile([NUM_IN + 1, 1024], BF16)
            nc.sync.dma_start(out=wnfb_sb[:], in_=wnfb_d[:])
            wnbb_sb = cpool.tile([NUM_IN + 1, 1024], BF16)
            nc.sync.dma_start(out=wnbb_sb[:], in_=wnbb_d[:])
            numt1_sb = cpool.tile([NUM_IN + 1, NT], BF16)
            nc.sync.dma_start(out=numt1_sb[:], in_=numt1_d[:])
            wo_sb = cpool.tile([128, 8], F32)
            nc.sync.dma_start(out=wo_sb[:], in_=wo_d[:])
            bo_sb = cpool.tile([1, 1], BF16)
            nc.sync.dma_start(out=bo_sb[:], in_=bo_d[:])

            # ---- embedding: indirect row-gather then PE transpose ----
            # per batch b: gather 4x[128 tok, 256] then transpose into
            # eT_b [128 d, 2, 512 tok] (k-chunk = emb dims k*128..)
            from concourse.masks import make_identity
            ident = cpool.tile([128, 128], BF16)
            make_identity(nc, ident[:])

            def build_eT(b):
                """gather + transpose batch b's embeddings -> eT_b [128,2,S]"""
                e_b = wpool.tile([128, 4, EMB], BF16, tag="eg")
                for g in range(4):
                    t0 = b * (S // 128) + g
                    nc.gpsimd.indirect_dma_start(
                        out=e_b[:, g, :],
                        out_offset=None,
                        in_=emb_d[:],
                        in_offset=bass.IndirectOffsetOnAxis(
                            ap=idx_sb[:, t0:t0 + 1], axis=0),
                    )
                eT_b = wpool.tile([128, 2, S], BF16, tag="eT")
                for k in range(2):
                    tp = pst.tile([128, 4, 128], BF16, tag="tp")
                    for g in range(4):
                        nc.tensor.transpose(
                            out=tp[:, g, :],
                            in_=e_b[:, g, k * 128:(k + 1) * 128],
                            identity=ident[:])
                    nc.vector.tensor_copy(out=eT_b[:, k, :], in_=tp[:])
                return eT_b

            def gate_mm3(out_ps, w_sb, wn_sb, col, rhs_e0, rhs_e1, rhs_n):
                # out_ps[h128, n] = sum_d W[d, col:col+128]^T ... (G^T chunk)
                nc.tensor.matmul(out_ps, lhsT=w_sb[:, col:col + 128],
                                 rhs=rhs_e0, start=True, stop=False)
                nc.tensor.matmul(out_ps, lhsT=w_sb[:, 1024 + col:1024 + col + 128],
                                 rhs=rhs_e1, start=False, stop=False)
                nc.tensor.matmul(out_ps, lhsT=wn_sb[:, col:col + 128],
                                 rhs=rhs_n, start=False, stop=True)

            # hS[h128, j, b]: forward final states; wtb: backward (s-1)*z
            hS = cpool.tile([128, 4, BC], F32)
            wtb = cpool.tile([128, 4, BC], F32)
            # last-token e^T columns for the backward direction
            eTlast = cpool.tile([128, 2, BC], BF16)

            # ---- forward direction: full-sequence gates + scan, per batch ----
            for b in range(BC):
                tok = slice(b * S, (b + 1) * S)
                eT_b = build_eT(b)
                nc.vector.tensor_copy(out=eTlast[:, :, b], in_=eT_b[:, :, S - 1])
                rhs_e0 = eT_b[:, 0, :]
                rhs_e1 = eT_b[:, 1, :]
                rhs_n = numt1_sb[:, tok]
                zps = ps.tile([128, 2, S], F32, tag="g")
                zps2 = ps.tile([128, 2, S], F32, tag="g")
                fps = ps.tile([128, 2, S], F32, tag="g")
                fps2 = ps.tile([128, 2, S], F32, tag="g")
                for j in range(4):
                    t = (zps, zps2)[j // 2][:, j % 2, :]
                    gate_mm3(t, wf_sb, wnfb_sb, j * 128, rhs_e0, rhs_e1, rhs_n)
                for j in range(4):
                    t = (fps, fps2)[j // 2][:, j % 2, :]
                    gate_mm3(t, wf_sb, wnfb_sb, 512 + j * 128, rhs_e0, rhs_e1, rhs_n)
                z_b = wpool.tile([128, 4, S], F32, tag="z")
                s_b = wpool.tile([128, 4, S], F32, tag="s")
                nc.scalar.activation(z_b[:, 0:2, :], zps[:], AF.Tanh)
                nc.scalar.activation(z_b[:, 2:4, :], zps2[:], AF.Tanh)
                nc.scalar.activation(s_b[:, 0:2, :], fps[:], AF.Sigmoid)
                nc.scalar.activation(s_b[:, 2:4, :], fps2[:], AF.Sigmoid)
                # w~ = (s - 1) * z ; then state = s*state - w~ == s*state + (1-s) z
                w_b = wpool.tile([128, 4, S], F32, tag="w")
                eng(STT_ENGINE).scalar_tensor_tensor(
                    out=w_b[:], in0=s_b[:], scalar=1.0, in1=z_b[:],
                    op0=ALU.subtract, op1=ALU.mult)
                h_b = wpool.tile([128, 4, S], F32, tag="h")
                for j in range(4):
                    eng(SCAN_ENGINES[j]).tensor_tensor_scan(
                        out=h_b[:, j, :], data0=s_b[:, j, :], data1=w_b[:, j, :],
                        initial=0.0, op0=ALU.mult, op1=ALU.subtract)
                nc.vector.tensor_copy(out=hS[:, :, b], in_=h_b[:, :, S - 1])

            # ---- backward direction: only t = S-1 matters ----
            rhs_e0 = eTlast[:, 0, :]          # [128, BC]
            rhs_e1 = eTlast[:, 1, :]
            rhs_n = numt1_sb[:, S - 1::S]     # [8, BC]
            zbps = ps.tile([128, 4, BC], F32, tag="g")
            fbps = ps.tile([128, 4, BC], F32, tag="g")
            for j in range(4):
                gate_mm3(zbps[:, j, :], wb_sb, wnbb_sb, j * 128, rhs_e0, rhs_e1, rhs_n)
            for j in range(4):
                gate_mm3(fbps[:, j, :], wb_sb, wnbb_sb, 512 + j * 128, rhs_e0, rhs_e1, rhs_n)
            zb_t = wpool.tile([128, 4, BC], F32, tag="zb")
            sb_t = wpool.tile([128, 4, BC], F32, tag="sb")
            nc.scalar.activation(zb_t[:], zbps[:], AF.Tanh)
            nc.scalar.activation(sb_t[:], fbps[:], AF.Sigmoid)
            nc.vector.scalar_tensor_tensor(
                out=wtb[:], in0=sb_t[:], scalar=1.0, in1=zb_t[:],
                op0=ALU.subtract, op1=ALU.mult)

            # ---- output projection: out[b] = sum_j hS[:,j,b].Wo_j - wtb[:,j,b].Wo_bj + bo
            # (wo columns 4..7 hold NEGATED backward Wo chunks, so plain accumulate)
            ops = ps.tile([BC, 1], F32, tag="g")
            for j in range(4):
                nc.tensor.matmul(ops[:], lhsT=hS[:, j, :], rhs=wo_sb[:, j:j + 1],
                                 start=(j == 0), stop=False)
            for j in range(4):
                nc.tensor.matmul(ops[:], lhsT=wtb[:, j, :], rhs=wo_sb[:, 4 + j:5 + j],
                                 start=False, stop=False)
            # + bo via a ones lhsT row
            ones_sb = cpool.tile([1, BC], BF16)
            nc.vector.memset(ones_sb[:], 1.0)
            nc.tensor.matmul(ops[:], lhsT=ones_sb[:],
                             rhs=bo_sb[:], start=False, stop=True)
            out_sb = cpool.tile([BC, 1], F32)
            nc.vector.tensor_copy(out=out_sb[:], in_=ops[:])
            nc.sync.dma_start(out=out_d[:], in_=out_sb[:])

    nc.compile()
    return nc


def prep_inputs(X, emb, Wn, bn, Wf, bf, Wb, bb, Wo, bo):
    """Host-side sharding + weight folding. Returns per-core input maps."""
    X = np.asarray(X, np.float32)
    emb = np.asarray(emb, np.float32)
    Wn = np.asarray(Wn, np.float32)
    bn = np.asarray(bn, np.float32)
    Wf = np.asarray(Wf, np.float32)
    bf_ = np.asarray(bf, np.float32)
    Wb = np.asarray(Wb, np.float32)
    bb_ = np.asarray(bb, np.float32)
    Wo = np.asarray(Wo, np.float32)
    bo_ = np.asarray(bo, np.float32)

    ev = X[:, :, 0].astype(np.int32)                       # [B,S]
    num = X[:, :, 1:]                                      # [B,S,7]

    def fold(W, bvec):
        Wzf = W[:, :2 * HID]                               # drop unused O gate
        w_emb = Wzf[:EMB]                                  # [256,1024]
        wf_resh = w_emb.reshape(2, 128, 2 * HID).transpose(1, 0, 2).reshape(128, 2 * 2 * HID)
        wnf = Wn @ Wzf[EMB:]                               # [7,1024]
        bias_eff = bvec[:2 * HID] + bn @ Wzf[EMB:]         # [1024]
        wnfb = np.concatenate([wnf, bias_eff[None, :]], axis=0)  # [8,1024]
        return wf_resh.astype(NP_BF16), wnfb.astype(NP_BF16)

    wf_resh, wnfb = fold(Wf, bf_)
    wb_resh, wnbb = fold(Wb, bb_)

    wo_resh = np.empty((128, 8), np.float32)
    for j in range(4):
        wo_resh[:, j] = Wo[j * 128:(j + 1) * 128, 0]
        wo_resh[:, 4 + j] = -Wo[HID + j * 128:HID + (j + 1) * 128, 0]

    emb_bf = emb.astype(NP_BF16)
    bo_bf = bo_.reshape(1, 1).astype(NP_BF16)

    in_maps = []
    for c in range(NCORES):
        bs = slice(c * BC, (c + 1) * BC)
        # token (b_local*S + g*128 + p) -> idx32[p, b_local*4+g]
        idx_flat = ev[bs].reshape(NT)                       # b-major tokens
        idx_wrapped = np.ascontiguousarray(idx_flat.reshape(NT // 128, 128).T)
        numt = num[bs].transpose(2, 0, 1).reshape(NUM_IN, NT)
        numt1 = np.concatenate([numt, np.ones((1, NT), np.float32)], 0).astype(NP_BF16)
        in_maps.append({
            "idx32": idx_wrapped,
            "numt1": numt1,
            "emb": emb_bf,
            "wf": wf_resh, "wnfb": wnfb,
            "wb": wb_resh, "wnbb": wnbb,
            "wo": wo_resh, "bo": bo_bf,
        })
    return in_maps


_NC_CACHE = {}


def kernel(X, emb, Wn, bn, Wf, bf, Wb, bb, Wo, bo):
    if "nc" not in _NC_CACHE:
        _NC_CACHE["nc"] = build_kernel()
    nc = _NC_CACHE["nc"]
    in_maps = prep_inputs(X, emb, Wn, bn, Wf, bf, Wb, bb, Wo, bo)
    res = bass_utils.run_bass_kernel_spmd(nc, in_maps, core_ids=list(range(NCORES)))
    return np.concatenate([res.results[c]["out"] for c in range(NCORES)], axis=0)
